# revision 8
# baseline (speedup 1.0000x reference)
"""NT-Xent contrastive loss (forward) on 8 TRN2 NeuronCores via Bass/Tile.

Math: with h = concat(h_i, h_j) [N=8192, D=256], sim = (h @ h.T) / 0.5,
loss = mean_r( logsumexp_j(sim[r, j], j != r) - pos_r ), where
pos_r = sim[r, partner(r)] = 2 * h_i[q] . h_j[q].  The loss separates:
loss = (sum_r lse_r - sum_r pos_r) / N, and sum_r pos_r = 4 * sum(h_i * h_j).

Sharding + symmetry: core c owns rows [1024c, 1024c+1024).  sim is
symmetric, so each core computes only 5 of the 8 column-blocks of its row
stripe (rotated columns [0, 5120)): block j=0 is its own diagonal block
(rowsums only, diagonal masked), blocks j=1..3 produce BOTH per-row sums
(this core's rows) and per-column sums (the mirror cores' rows, shipped to
the host which assembles S_r from all cores), and block j=4 is computed by
both cores of the pair, rowsums only.  Column sums cost no DVE/ACT work:
the PE accumulates ones.T @ exp(tile) into persistent PSUM accumulators
across all 8 row-blocks (32-partition col-tiling packs 4 accumulators per
PSUM bank).

Engines: fp8 e4m3 inputs, each 512-col sim chunk is ONE DoubleRow matmul
(K=256 packed, ~2x bf16 PE rate).  Per row-block of 128: ACT exponentiates
cols [1024:3072) (to SBUF bf16, fused row-accum) and [4096:5120)
(in-place); the DVE handles [0:1024) and [3072:4096) with a bf16/u16
Schraudolph fast-exp (+-4% per term, phase-averaged ~1e-5) followed by an
in-place identity tensor_scalar whose accum_out yields the row sum at the
DVE's 4x 16-bit rate.  gpsimd does the positive-pair dots.  The host
finishes with log/sum in float64.  fp8 quantization dominates the error:
~9e-4 relative (tolerance 2e-2).
"""

import numpy as np
import ml_dtypes

B = 4096
D = 256
N = 2 * B            # 8192 rows/cols of sim
NCORES = 8
RPC = N // NCORES    # 1024 rows per core
KCH = D // 128       # 2 contraction chunks of 128
NRB = RPC // 128     # 8 row-blocks of 128 per core
NCOL = 5120          # rotated columns computed per core (blocks j=0..4)
M_DEFAULT = 161.0    # logsumexp shift; safe while rowmax(2*h@h.T) in [M-70, M+79]
MASK_NEG = -1.0e9

# Schraudolph fast-exp constants, bf16/u16 variant:
#   bits16 = round(x * 2*A16 + (B16 - A16*M)), bitcast u16 -> bf16.
EXP_A16 = float(2 ** 7 / np.log(2.0))
EXP_B16 = 1064865216.0 / 65536.0

TRACE = False        # set True (e.g. from test.py) to request an NTFF trace
LAST_RESULTS = None  # BassKernelResults of the last run (for profiling)

_cache = {}


def _build():
    """Build the SPMD Bass/Tile program once per process."""
    if "nc" in _cache:
        return _cache["nc"]

    import concourse.tile as tile
    import concourse.mybir as mybir
    from concourse import bacc

    f32 = mybir.dt.float32
    bf16 = mybir.dt.bfloat16
    f8 = mybir.dt.float8e4
    u16 = mybir.dt.uint16
    DR = mybir.MatmulPerfMode.DoubleRow
    ALU = mybir.AluOpType

    nc = bacc.Bacc("TRN2", target_bir_lowering=False, num_devices=NCORES)
    ht_dram = nc.dram_tensor("ht", [KCH, 128, NCOL], f8, kind="ExternalInput").ap()
    eye_dram = nc.dram_tensor("eye", [1, 128, 128], bf16, kind="ExternalInput").ap()
    maskr_dram = nc.dram_tensor("maskr", [128, 4, 512], bf16, kind="ExternalInput").ap()
    bias_dram = nc.dram_tensor("biasm", [128, NRB], f32, kind="ExternalInput").ap()
    bias2_dram = nc.dram_tensor("bias2", [128, NRB], f32, kind="ExternalInput").ap()
    out_dram = nc.dram_tensor("out", [128, 36], f32, kind="ExternalOutput").ap()
    acc_dram = nc.dram_tensor("outacc", [6, 512], f32, kind="ExternalOutput").ap()

    with tile.TileContext(nc) as tc:
        with (
            tc.tile_pool(name="hpool", bufs=1) as hpool,
            tc.tile_pool(name="small", bufs=1) as small,
            tc.tile_pool(name="epool", bufs=2) as epool,
            tc.tile_pool(name="ipool", bufs=3) as ipool,
            tc.tile_pool(name="psring", bufs=1, space="PSUM") as psring,
            tc.tile_pool(name="psacc", bufs=1, space="PSUM") as psacc,
        ):
            # Small constants on the gpsimd (SWDGE) queue while the sync
            # queue streams the big h.T chunks.
            eye_pos = small.tile([128, 128], bf16)
            nc.gpsimd.dma_start(out=eye_pos, in_=eye_dram[0])
            maskr_sb = small.tile([128, 4, 512], bf16)
            nc.gpsimd.dma_start(out=maskr_sb, in_=maskr_dram)
            bias_sb = small.tile([128, NRB], f32)
            nc.gpsimd.dma_start(out=bias_sb, in_=bias_dram)
            bias2_sb = small.tile([128, NRB], f32)
            nc.gpsimd.dma_start(out=bias2_sb, in_=bias2_dram)

            ones_sb = small.tile([128, 1], bf16)
            nc.vector.memset(ones_sb, 1.0)

            # Warm the ACT exp table during the DMA prologue.
            warm_sb = small.tile([128, 1], f32)
            nc.scalar.activation(
                out=warm_sb, in_=bias_sb[:, 0:1],
                func=mybir.ActivationFunctionType.Exp, bias=0.0, scale=0.0,
            )

            # 6-bank PSUM ring for the streamed sim chunks + 2 banks of
            # persistent column-sum accumulators.
            ring = psring.tile([128, 3072], f32, name="ring")
            accA = psacc.tile([128, 512], f32, name="accA")
            accB = psacc.tile([128, 512], f32, name="accB")

            # Bridge the PE-idle window before the first h.T chunk lands so
            # the HAM clock gate sees sustained activity early.
            wsrc = small.tile([128, 128], bf16)
            nc.vector.memset(wsrc, 0.0)
            for w in range(8):
                nc.tensor.matmul(
                    ring[:, 2560 + (w % 4) * 128:2560 + (w % 4) * 128 + 128],
                    lhsT=wsrc, rhs=wsrc, start=True, stop=True,
                )

            # h.T in SBUF in consumption order; tile layout [128, 2, width]
            # is exactly the DoubleRow access-pattern shape.
            col_ranges = [(0, 512), (512, 1024), (1024, 2048),
                          (2048, 3072), (3072, 4096), (4096, 5120)]
            ht_tiles = []
            for c0, c1 in col_ranges:
                t = hpool.tile([128, KCH, c1 - c0], f8, name=f"ht_{c0}")
                nc.sync.dma_start(
                    out=t,
                    in_=ht_dram[:, :, c0:c1].rearrange("k p c -> p k c"),
                )
                ht_tiles.append(t)

            def rhs_slice(c0, w=512):
                """[128, 2, w] slice of rotated h.T at global column c0."""
                for (r0, r1), t in zip(col_ranges, ht_tiles):
                    if r0 <= c0 < r1:
                        assert c0 + w <= r1
                        return t[:, :, c0 - r0:c0 - r0 + w]
                raise AssertionError(c0)

            def lhsT_dr(rb):
                """[128, 2, 128] row-block weights (columns rb*128..+128)."""
                t = ht_tiles[0] if rb < 4 else ht_tiles[1]
                o = (rb % 4) * 128
                return t[:, :, o:o + 128]

            res_sb = small.tile([128, 36], f32)

            cursor = [0]  # ring position, 512-col units mod 6

            def ring_slice(units):
                off = cursor[0] * 512
                assert cursor[0] + units <= 6
                cursor[0] = (cursor[0] + units) % 6
                return ring[:, off:off + units * 512]

            def emit_Dj0(rb):
                # Own diagonal block, cols [0:1024): Schraudolph + rowsum,
                # diagonal masked via the accumulating I.T @ maskr matmul.
                ps = ring_slice(2)
                cs0 = rb // 4
                for cs in range(2):
                    nc.tensor.matmul(
                        ps[:, cs * 512:(cs + 1) * 512],
                        lhsT=lhsT_dr(rb), rhs=rhs_slice(cs * 512),
                        start=True, stop=cs != cs0, perf_mode=DR,
                    )
                nc.tensor.matmul(
                    ps[:, cs0 * 512:(cs0 + 1) * 512],
                    lhsT=eye_pos, rhs=maskr_sb[:, rb % 4, :],
                    start=False, stop=True,
                )
                ti = ipool.tile([128, 1024], u16, name="ti")
                nc.vector.tensor_scalar(
                    ti, ps, 2.0 * EXP_A16, bias2_sb[:, rb:rb + 1],
                    ALU.mult, ALU.add,
                )
                tb = ti.bitcast(bf16)
                nc.vector.tensor_scalar(
                    tb, tb, 0.0, 0.0, ALU.add, ALU.add,
                    accum_out=res_sb[:, rb * 4:rb * 4 + 1],
                )

            def emit_A(rb):
                # Cols [1024:3072): ACT exp -> SBUF bf16 + fused rowsum;
                # PE accumulates the four 512-col column sums into accA.
                ps = ring_slice(4)
                for cs in range(4):
                    nc.tensor.matmul(
                        ps[:, cs * 512:(cs + 1) * 512],
                        lhsT=lhsT_dr(rb), rhs=rhs_slice(1024 + cs * 512),
                        start=True, stop=True, perf_mode=DR,
                    )
                eA = epool.tile([128, 2048], bf16, name="eA")
                nc.scalar.activation(
                    out=eA, in_=ps,
                    func=mybir.ActivationFunctionType.Exp,
                    bias=bias_sb[:, rb:rb + 1], scale=2.0,
                    accum_out=res_sb[:, rb * 4 + 1:rb * 4 + 2],
                )
                for q in range(4):
                    nc.tensor.matmul(
                        accA[32 * q:32 * q + 1, :],
                        lhsT=ones_sb, rhs=eA[:, 512 * q:512 * (q + 1)],
                        start=rb == 0, stop=rb == NRB - 1,
                        tile_position=(0, 32 * q), skip_group_check=True,
                    )

            def emit_Dj3(rb):
                # Cols [3072:4096): Schraudolph + rowsum on DVE; PE
                # accumulates the two column sums into accB.
                ps = ring_slice(2)
                for cs in range(2):
                    nc.tensor.matmul(
                        ps[:, cs * 512:(cs + 1) * 512],
                        lhsT=lhsT_dr(rb), rhs=rhs_slice(3072 + cs * 512),
                        start=True, stop=True, perf_mode=DR,
                    )
                ti = ipool.tile([128, 1024], u16, name="ti")
                nc.vector.tensor_scalar(
                    ti, ps, 2.0 * EXP_A16, bias2_sb[:, rb:rb + 1],
                    ALU.mult, ALU.add,
                )
                tb = ti.bitcast(bf16)
                nc.vector.tensor_scalar(
                    tb, tb, 0.0, 0.0, ALU.add, ALU.add,
                    accum_out=res_sb[:, rb * 4 + 2:rb * 4 + 3],
                )
                for q in range(2):
                    nc.tensor.matmul(
                        accB[32 * q:32 * q + 1, :],
                        lhsT=ones_sb, rhs=tb[:, 512 * q:512 * (q + 1)],
                        start=rb == 0, stop=rb == NRB - 1,
                        tile_position=(0, 32 * q), skip_group_check=True,
                    )

            def emit_Aj4(rb):
                # Cols [4096:5120): the pair-duplicated block; rowsums only,
                # ACT exp in place.
                ps = ring_slice(2)
                for cs in range(2):
                    nc.tensor.matmul(
                        ps[:, cs * 512:(cs + 1) * 512],
                        lhsT=lhsT_dr(rb), rhs=rhs_slice(4096 + cs * 512),
                        start=True, stop=True, perf_mode=DR,
                    )
                nc.scalar.activation(
                    out=ps, in_=ps,
                    func=mybir.ActivationFunctionType.Exp,
                    bias=bias_sb[:, rb:rb + 1], scale=2.0,
                    accum_out=res_sb[:, rb * 4 + 3:rb * 4 + 4],
                )

            def emit_posdot():
                # Positive-pair partial dots: rotated cols [0:1024) are this
                # core's rows, [4096:5120) their partners.  gpsimd multiply
                # (SBUF only), DVE 16-bit accum pass for the row sums; split
                # per 512 since cols [0:1024) span two SBUF tiles.
                for k in range(KCH):
                    for half in range(2):
                        pp = small.tile([128, 512], bf16, name=f"pp_{k}_{half}")
                        nc.gpsimd.tensor_mul(
                            pp,
                            ht_tiles[half][:, k, :],
                            ht_tiles[5][:, k, half * 512:(half + 1) * 512],
                        )
                        nc.vector.tensor_scalar(
                            pp, pp, 0.0, 0.0, ALU.add, ALU.add,
                            accum_out=res_sb[:, 32 + 2 * k + half:33 + 2 * k + half],
                        )

            EMIT = {"0": emit_Dj0, "A": emit_A, "3": emit_Dj3, "4": emit_Aj4}
            for rb in range(NRB):
                if rb == 5:
                    emit_posdot()
                # Ring-fit chunk order: the 4-unit A chunk must not wrap the
                # 6-unit ring, so every third row-block emits A first.
                order = "A034" if cursor[0] == 2 else "0A34"
                for ch in order:
                    EMIT[ch](rb)

            # Ship rb0-6 partials + posdots while rb7 still computes.
            nc.sync.dma_start(out=out_dram[:, 0:28], in_=res_sb[:, 0:28])

            # Column-sum accumulators: PSUM -> SBUF (DMA cannot read PSUM),
            # then one tiny DMA of the 6 used partition rows.
            accA_sb = small.tile([128, 512], f32)
            accB_sb = small.tile([128, 512], f32)
            nc.scalar.copy(accA_sb, accA)
            nc.vector.tensor_copy(accB_sb, accB)
            nc.sync.dma_start(out=acc_dram[0:4, :], in_=accA_sb[0:128:32, :])
            nc.sync.dma_start(out=acc_dram[4:6, :], in_=accB_sb[0:64:32, :])
            nc.sync.dma_start(out=out_dram[:, 28:36], in_=res_sb[:, 28:36])

    nc.compile()
    _cache["nc"] = nc
    return nc


def _make_static_inputs(h_i, h_j):
    """Per-core rotated h.T (fp8 e4m3), cols [0:5120) only, plus masks."""
    h = np.concatenate([np.asarray(h_i), np.asarray(h_j)], axis=0).astype(np.float32)
    hT = np.ascontiguousarray(h.T)  # [256, 8192]
    hts = []
    for c in range(NCORES):
        htc = np.roll(hT, -RPC * c, axis=1)[:, :NCOL]
        hts.append(
            np.ascontiguousarray(htc.astype(ml_dtypes.float8_e4m3).reshape(KCH, 128, NCOL))
        )
    eye = np.zeros((1, 128, 128), dtype=ml_dtypes.bfloat16)
    p = np.arange(128)
    eye[0, p, p] = 1.0
    maskr = np.zeros((128, 4, 512), dtype=ml_dtypes.bfloat16)
    for v in range(4):
        maskr[p, v, 128 * v + p] = MASK_NEG
    return hts, eye, maskr


def _axon_reset():
    try:
        import ctypes

        lib = ctypes.CDLL("/opt/axon/libaxon_pjrt.so")
        lib.axon_reset.restype = ctypes.c_int64
        return lib.axon_reset() == 0
    except Exception:
        return False


def _run(nc, hts, eye, maskr, M):
    global LAST_RESULTS
    from concourse import bass_utils

    in_maps = [
        {
            "ht": hts[c],
            "eye": eye,
            "maskr": maskr,
            "biasm": np.full((128, NRB), -M, dtype=np.float32),
            "bias2": np.full((128, NRB), EXP_B16 - EXP_A16 * M, dtype=np.float32),
        }
        for c in range(NCORES)
    ]
    try:
        results = bass_utils.run_bass_kernel_spmd(
            nc, in_maps, core_ids=list(range(NCORES)), trace=TRACE
        )
    except Exception:
        if not _axon_reset():
            raise
        results = bass_utils.run_bass_kernel_spmd(
            nc, in_maps, core_ids=list(range(NCORES)), trace=TRACE
        )
    LAST_RESULTS = results
    return results.results


def _host_fallback(h_i, h_j):
    """Exact float64 evaluation on the host (safety net for data far
    outside the M window; never triggered by in-distribution inputs)."""
    h = np.concatenate([np.asarray(h_i), np.asarray(h_j)], 0).astype(np.float64)
    sim = 2.0 * (h @ h.T)
    np.fill_diagonal(sim, -np.inf)
    m = sim.max(1)
    lse = m + np.log(np.exp(sim - m[:, None]).sum(1))
    pos = np.concatenate([2.0 * (h[:B] * h[B:]).sum(1)] * 2)
    return np.float32((lse - pos).mean())


def kernel(h_i, h_j):
    nc = _build()
    hts, eye, maskr = _make_static_inputs(h_i, h_j)

    for attempt, M in enumerate([M_DEFAULT, M_DEFAULT - 60.0, M_DEFAULT + 60.0]):
        res = _run(nc, hts, eye, maskr, M)
        # Assemble per-row exp sums: own row partials + mirror column sums.
        S = np.zeros(N)
        total_pd = 0.0
        for c in range(NCORES):
            out = res[c]["out"].astype(np.float64)
            acc = res[c]["outacc"].astype(np.float64)
            own = out[:, :32].reshape(128, NRB, 4).sum(axis=2)  # [p, rb]
            rows = (RPC * c + np.arange(RPC)) % N
            S[rows] += own.T.reshape(RPC)
            for q in range(6):
                j0 = 1024 + 512 * q
                tgt = (RPC * c + j0 + np.arange(512)) % N
                S[tgt] += acc[q]
            total_pd += out[:, 32:36].sum()
        if np.all(np.isfinite(S) & (S > 0.0)):
            total_lse = N * M + np.log(S).sum()
            loss = (total_lse - 2.0 * total_pd) / float(N)
            return np.array(loss, dtype=np.float32)

    return _host_fallback(h_i, h_j)


if __name__ == "__main__":
    rng = np.random.default_rng(0)
    h_i = rng.standard_normal((B, D), dtype=np.float32)
    h_j = rng.standard_normal((B, D), dtype=np.float32)
    print("loss:", kernel(h_i, h_j))


# revision 9
# speedup vs baseline: 1.7561x; 1.7561x over previous
"""NT-Xent contrastive loss (forward) on 8 TRN2 NeuronCores via Bass/Tile.

Math: with h = concat(h_i, h_j) [N=8192, D=256], sim = (h @ h.T) / 0.5,
loss = mean_r( logsumexp_j(sim[r, j], j != r) - pos_r ), where
pos_r = sim[r, partner(r)] = 2 * h_i[q] . h_j[q].  The loss separates:
loss = (sum_r lse_r - sum_r pos_r) / N, and sum_r pos_r = 4 * sum(h_i * h_j).

Sharding + symmetry: core c owns rows [1024c, 1024c+1024).  sim is
symmetric, so each core computes only 5 of the 8 column-blocks of its row
stripe (rotated columns [0, 5120)): block j=0 is its own diagonal block
(rowsums only, diagonal masked), blocks j=1..3 produce BOTH per-row sums
(this core's rows) and per-column sums (the mirror cores' rows; host
assembles S_r from all cores), and block j=4 is computed by both cores of
its pair, rowsums only.  Column sums cost no ACT/DVE work: the PE
accumulates ones.T @ exp(tile) into persistent PSUM accumulators across
all 8 row-blocks, 4 accumulators packed per PSUM bank via 32-partition
col-tiling (4 such matmuls run concurrently in distinct array
col-groups).  The column-sum matmuls for row-block rb are emitted after
rb+1's sim matmuls so the PE never waits on ACT.

Engines: fp8 e4m3 inputs; each 512-col sim chunk is ONE DoubleRow matmul
(K=256 packed, ~2x bf16 PE rate).  Per 128-row block: ACT exponentiates
cols [1024:4096) (three 1024-col chunks -> SBUF bf16 + fused row-accum);
DVE handles [0:1024) and [4096:5120) with a bf16/u16 Schraudolph fast-exp
(+-4% per term, phase-averaged ~1e-5); gpsimd folds each Schraudolph tile
1024->512->256 (fp32 adds) so the DVE row-reduce touches only 256 cols.
gpsimd also does the positive-pair dots.  The host finishes with log/sum
in float64.  fp8 quantization dominates the error: ~9e-4 relative
(tolerance 2e-2).
"""

import numpy as np
import ml_dtypes

B = 4096
D = 256
N = 2 * B            # 8192 rows/cols of sim
NCORES = 8
RPC = N // NCORES    # 1024 rows per core
KCH = D // 128       # 2 contraction chunks of 128
NRB = RPC // 128     # 8 row-blocks of 128 per core
NCOL = 5120          # rotated columns computed per core (blocks j=0..4)
M_DEFAULT = 161.0    # logsumexp shift; safe while rowmax(2*h@h.T) in [M-70, M+79]
MASK_NEG = -1.0e9

# Schraudolph fast-exp constants, bf16/u16 variant:
#   bits16 = round(x * 2*A16 + (B16 - A16*M)), bitcast u16 -> bf16.
EXP_A16 = float(2 ** 7 / np.log(2.0))
EXP_B16 = 1064865216.0 / 65536.0

TRACE = False        # set True (e.g. from test.py) to request an NTFF trace
LAST_RESULTS = None  # BassKernelResults of the last run (for profiling)

_cache = {}


def _build():
    """Build the SPMD Bass/Tile program once per process."""
    if "nc" in _cache:
        return _cache["nc"]

    import concourse.tile as tile
    import concourse.mybir as mybir
    from concourse import bacc

    f32 = mybir.dt.float32
    bf16 = mybir.dt.bfloat16
    f8 = mybir.dt.float8e4
    u16 = mybir.dt.uint16
    DR = mybir.MatmulPerfMode.DoubleRow
    ALU = mybir.AluOpType
    AX = mybir.AxisListType.X

    nc = bacc.Bacc("TRN2", target_bir_lowering=False, num_devices=NCORES)
    ht_dram = nc.dram_tensor("ht", [KCH, 128, NCOL], f8, kind="ExternalInput").ap()
    eye_dram = nc.dram_tensor("eye", [1, 128, 128], bf16, kind="ExternalInput").ap()
    maskr_dram = nc.dram_tensor("maskr", [128, 4, 512], bf16, kind="ExternalInput").ap()
    bias_dram = nc.dram_tensor("biasm", [128, NRB], f32, kind="ExternalInput").ap()
    bias2_dram = nc.dram_tensor("bias2", [128, NRB], f32, kind="ExternalInput").ap()
    out_dram = nc.dram_tensor("out", [128, 44], f32, kind="ExternalOutput").ap()
    acc_dram = nc.dram_tensor("outacc", [6, 512], f32, kind="ExternalOutput").ap()

    with tile.TileContext(nc) as tc:
        with (
            tc.tile_pool(name="hpool", bufs=1) as hpool,
            tc.tile_pool(name="small", bufs=1) as small,
            tc.tile_pool(name="epool", bufs=6) as epool,
            tc.tile_pool(name="ipool", bufs=3) as ipool,
            tc.tile_pool(name="fpool", bufs=3) as fpool,
            tc.tile_pool(name="pschunk", bufs=3, space="PSUM") as pschunk,
            tc.tile_pool(name="psacc", bufs=1, space="PSUM") as psacc,
        ):
            # Small constants on the gpsimd (SWDGE) queue while the sync
            # queue streams the big h.T chunks.
            eye_pos = small.tile([128, 128], bf16)
            nc.gpsimd.dma_start(out=eye_pos, in_=eye_dram[0])
            maskr_sb = small.tile([128, 4, 512], bf16)
            nc.gpsimd.dma_start(out=maskr_sb, in_=maskr_dram)
            bias_sb = small.tile([128, NRB], f32)
            nc.gpsimd.dma_start(out=bias_sb, in_=bias_dram)
            bias2_sb = small.tile([128, NRB], f32)
            nc.gpsimd.dma_start(out=bias2_sb, in_=bias2_dram)

            ones_sb = small.tile([128, 1], bf16)
            nc.vector.memset(ones_sb, 1.0)

            # Warm the ACT exp table during the DMA prologue.
            warm_sb = small.tile([128, 1], f32)
            nc.scalar.activation(
                out=warm_sb, in_=bias_sb[:, 0:1],
                func=mybir.ActivationFunctionType.Exp, bias=0.0, scale=0.0,
            )

            accA = psacc.tile([128, 512], f32, name="accA")
            accB = psacc.tile([128, 512], f32, name="accB")

            # Bridge the PE-idle window before the first h.T chunk lands so
            # the HAM clock gate sees sustained activity early.
            wsrc = small.tile([128, 128], bf16)
            nc.vector.memset(wsrc, 0.0)
            wps = pschunk.tile([128, 1024], f32, name="ps")
            for w in range(8):
                nc.tensor.matmul(
                    wps[:, (w % 4) * 128:(w % 4) * 128 + 128],
                    lhsT=wsrc, rhs=wsrc, start=True, stop=True,
                )

            # h.T in SBUF in consumption order; tile layout [128, 2, width]
            # is exactly the DoubleRow access-pattern shape.
            col_ranges = [(0, 512), (512, 1024), (1024, 2048),
                          (2048, 3072), (3072, 4096), (4096, 5120)]
            ht_tiles = []
            for c0, c1 in col_ranges:
                t = hpool.tile([128, KCH, c1 - c0], f8, name=f"ht_{c0}")
                nc.sync.dma_start(
                    out=t,
                    in_=ht_dram[:, :, c0:c1].rearrange("k p c -> p k c"),
                )
                ht_tiles.append(t)

            def rhs_slice(c0, w=512):
                """[128, 2, w] slice of rotated h.T at global column c0."""
                for (r0, r1), t in zip(col_ranges, ht_tiles):
                    if r0 <= c0 < r1:
                        assert c0 + w <= r1
                        return t[:, :, c0 - r0:c0 - r0 + w]
                raise AssertionError(c0)

            def lhsT_dr(rb):
                """[128, 2, 128] row-block weights (columns rb*128..+128)."""
                t = ht_tiles[0] if rb < 4 else ht_tiles[1]
                o = (rb % 4) * 128
                return t[:, :, o:o + 128]

            res_sb = small.tile([128, 44], f32)

            def sim_mms(rb, c0, ps, mask=False):
                """Two DoubleRow matmuls filling ps[128,1024] = sim cols
                [c0, c0+1024); optionally accumulate the diag mask."""
                cs0 = rb // 4
                for cs in range(2):
                    nc.tensor.matmul(
                        ps[:, cs * 512:(cs + 1) * 512],
                        lhsT=lhsT_dr(rb), rhs=rhs_slice(c0 + cs * 512),
                        start=True, stop=not (mask and cs == cs0),
                        perf_mode=DR,
                    )
                if mask:
                    nc.tensor.matmul(
                        ps[:, cs0 * 512:(cs0 + 1) * 512],
                        lhsT=eye_pos, rhs=maskr_sb[:, rb % 4, :],
                        start=False, stop=True,
                    )

            def emit_D(rb, c0, rescol):
                # Schraudolph chunk: DVE fast-exp bits, gpsimd folds
                # 1024->512->256 in fp32, DVE row-reduces the 256.
                ps = pschunk.tile([128, 1024], f32, name="ps")
                sim_mms(rb, c0, ps, mask=c0 == 0)
                ti = ipool.tile([128, 1024], u16, name="ti")
                nc.vector.tensor_scalar(
                    ti, ps, 2.0 * EXP_A16, bias2_sb[:, rb:rb + 1],
                    ALU.mult, ALU.add,
                )
                tb = ti.bitcast(bf16)
                f1 = fpool.tile([128, 512], f32, name="f1")
                nc.gpsimd.tensor_add(f1, tb[:, 0:512], tb[:, 512:1024])
                f2 = fpool.tile([128, 256], f32, name="f2")
                nc.gpsimd.tensor_add(f2, f1[:, 0:256], f1[:, 256:512])
                nc.vector.reduce_sum(res_sb[:, rescol:rescol + 1], f2, axis=AX)

            def emit_A(rb, c0, rescol):
                # ACT chunk: exp -> SBUF bf16 (feeds the delayed column-sum
                # matmuls) + fused row-accum.
                ps = pschunk.tile([128, 1024], f32, name="ps")
                sim_mms(rb, c0, ps)
                eA = epool.tile([128, 1024], bf16, name="eA")
                nc.scalar.activation(
                    out=eA, in_=ps,
                    func=mybir.ActivationFunctionType.Exp,
                    bias=bias_sb[:, rb:rb + 1], scale=2.0,
                    accum_out=res_sb[:, rescol:rescol + 1],
                )
                return eA

            def emit_colsums(rb, eAs):
                # Column sums of exp for cols [1024:4096) of row-block rb:
                # six ones.T @ eA matmuls accumulating into accA (4 slots)
                # and accB (2 slots); concurrent within distinct col-groups.
                for g in range(6):
                    acc, q = (accA, g) if g < 4 else (accB, g - 4)
                    nc.tensor.matmul(
                        acc[32 * q:32 * q + 1, :],
                        lhsT=ones_sb,
                        rhs=eAs[g // 2][:, (g % 2) * 512:(g % 2) * 512 + 512],
                        start=rb == 0, stop=rb == NRB - 1,
                        tile_position=(0, 32 * q), skip_group_check=True,
                    )

            def emit_posdot():
                # Positive-pair partial dots: rotated cols [0:1024) are this
                # core's rows, [4096:5120) their partners.  gpsimd multiply
                # and fold, tiny DVE row-reduce.
                for k in range(KCH):
                    for half in range(2):
                        pp = small.tile([128, 512], f32, name=f"pp_{k}_{half}")
                        nc.gpsimd.tensor_mul(
                            pp,
                            ht_tiles[half][:, k, :],
                            ht_tiles[5][:, k, half * 512:(half + 1) * 512],
                        )
                        nc.vector.reduce_sum(
                            res_sb[:, 40 + 2 * k + half:41 + 2 * k + half],
                            pp, axis=AX,
                        )

            prev_eAs = None
            for rb in range(NRB):
                emit_D(rb, 0, rb * 5 + 0)
                eA1 = emit_A(rb, 1024, rb * 5 + 1)
                if prev_eAs is not None:
                    emit_colsums(rb - 1, prev_eAs)
                eA2 = emit_A(rb, 2048, rb * 5 + 2)
                eA3 = emit_A(rb, 3072, rb * 5 + 3)
                if rb == 2:
                    emit_posdot()
                emit_D(rb, 4096, rb * 5 + 4)
                prev_eAs = (eA1, eA2, eA3)
            emit_colsums(NRB - 1, prev_eAs)

            # Ship rb0-6 partials + posdots while rb7 still computes.
            nc.sync.dma_start(out=out_dram[:, 0:35], in_=res_sb[:, 0:35])

            # Column-sum accumulators: PSUM -> SBUF (DMA cannot read PSUM),
            # then one tiny DMA of the 6 used partition rows.
            accA_sb = small.tile([128, 512], f32)
            accB_sb = small.tile([128, 512], f32)
            nc.scalar.copy(accA_sb, accA)
            nc.vector.tensor_copy(accB_sb, accB)
            nc.sync.dma_start(out=acc_dram[0:4, :], in_=accA_sb[0:128:32, :])
            nc.sync.dma_start(out=acc_dram[4:6, :], in_=accB_sb[0:64:32, :])
            nc.sync.dma_start(out=out_dram[:, 35:44], in_=res_sb[:, 35:44])

    nc.compile()
    _cache["nc"] = nc
    return nc


def _make_static_inputs(h_i, h_j):
    """Per-core rotated h.T (fp8 e4m3), cols [0:5120) only, plus masks."""
    h = np.concatenate([np.asarray(h_i), np.asarray(h_j)], axis=0).astype(np.float32)
    hT = np.ascontiguousarray(h.T)  # [256, 8192]
    hts = []
    for c in range(NCORES):
        htc = np.roll(hT, -RPC * c, axis=1)[:, :NCOL]
        hts.append(
            np.ascontiguousarray(htc.astype(ml_dtypes.float8_e4m3).reshape(KCH, 128, NCOL))
        )
    eye = np.zeros((1, 128, 128), dtype=ml_dtypes.bfloat16)
    p = np.arange(128)
    eye[0, p, p] = 1.0
    maskr = np.zeros((128, 4, 512), dtype=ml_dtypes.bfloat16)
    for v in range(4):
        maskr[p, v, 128 * v + p] = MASK_NEG
    return hts, eye, maskr


def _axon_reset():
    try:
        import ctypes

        lib = ctypes.CDLL("/opt/axon/libaxon_pjrt.so")
        lib.axon_reset.restype = ctypes.c_int64
        return lib.axon_reset() == 0
    except Exception:
        return False


def _run(nc, hts, eye, maskr, M):
    global LAST_RESULTS
    from concourse import bass_utils

    in_maps = [
        {
            "ht": hts[c],
            "eye": eye,
            "maskr": maskr,
            "biasm": np.full((128, NRB), -M, dtype=np.float32),
            "bias2": np.full((128, NRB), EXP_B16 - EXP_A16 * M, dtype=np.float32),
        }
        for c in range(NCORES)
    ]
    try:
        results = bass_utils.run_bass_kernel_spmd(
            nc, in_maps, core_ids=list(range(NCORES)), trace=TRACE
        )
    except Exception:
        if not _axon_reset():
            raise
        results = bass_utils.run_bass_kernel_spmd(
            nc, in_maps, core_ids=list(range(NCORES)), trace=TRACE
        )
    LAST_RESULTS = results
    return results.results


def _host_fallback(h_i, h_j):
    """Exact float64 evaluation on the host (safety net for data far
    outside the M window; never triggered by in-distribution inputs)."""
    h = np.concatenate([np.asarray(h_i), np.asarray(h_j)], 0).astype(np.float64)
    sim = 2.0 * (h @ h.T)
    np.fill_diagonal(sim, -np.inf)
    m = sim.max(1)
    lse = m + np.log(np.exp(sim - m[:, None]).sum(1))
    pos = np.concatenate([2.0 * (h[:B] * h[B:]).sum(1)] * 2)
    return np.float32((lse - pos).mean())


def kernel(h_i, h_j):
    nc = _build()
    hts, eye, maskr = _make_static_inputs(h_i, h_j)

    for attempt, M in enumerate([M_DEFAULT, M_DEFAULT - 60.0, M_DEFAULT + 60.0]):
        res = _run(nc, hts, eye, maskr, M)
        # Assemble per-row exp sums: own row partials + mirror column sums.
        S = np.zeros(N)
        total_pd = 0.0
        for c in range(NCORES):
            out = res[c]["out"].astype(np.float64)
            acc = res[c]["outacc"].astype(np.float64)
            own = out[:, :40].reshape(128, NRB, 5).sum(axis=2)  # [p, rb]
            rows = (RPC * c + np.arange(RPC)) % N
            S[rows] += own.T.reshape(RPC)
            for q in range(6):
                j0 = 1024 + 512 * q
                tgt = (RPC * c + j0 + np.arange(512)) % N
                S[tgt] += acc[q]
            total_pd += out[:, 40:44].sum()
        if np.all(np.isfinite(S) & (S > 0.0)):
            total_lse = N * M + np.log(S).sum()
            loss = (total_lse - 2.0 * total_pd) / float(N)
            return np.array(loss, dtype=np.float32)

    return _host_fallback(h_i, h_j)


if __name__ == "__main__":
    rng = np.random.default_rng(0)
    h_i = rng.standard_normal((B, D), dtype=np.float32)
    h_j = rng.standard_normal((B, D), dtype=np.float32)
    print("loss:", kernel(h_i, h_j))


# revision 11
# speedup vs baseline: 1.9924x; 1.1345x over previous
"""NT-Xent contrastive loss (forward) on 8 TRN2 NeuronCores via Bass/Tile.

Math: with h = concat(h_i, h_j) [N=8192, D=256], sim = (h @ h.T) / 0.5,
loss = mean_r( logsumexp_j(sim[r, j], j != r) - pos_r ), where
pos_r = sim[r, partner(r)] = 2 * h_i[q] . h_j[q].  The loss separates:
loss = (sum_r lse_r - sum_r pos_r) / N, and sum_r pos_r = 4 * sum(h_i * h_j).

Sharding + symmetry: core c owns rows [1024c, 1024c+1024).  sim is
symmetric, so each core computes only 5 of the 8 column-blocks of its row
stripe (rotated columns [0, 5120)): block j=0 is its own diagonal block
(rowsums only, diagonal masked), blocks j=1..3 produce BOTH per-row sums
(this core's rows) and per-column sums (the mirror cores' rows; host
assembles S_r from all cores), and block j=4 is computed by both cores of
its pair, rowsums only.  Column sums cost no ACT/DVE work: the PE
accumulates ones.T @ exp(tile) into persistent PSUM accumulators across
all 8 row-blocks, 4 accumulators packed per PSUM bank via 32-partition
col-tiling (4 such matmuls run concurrently in distinct array
col-groups).  The column-sum matmuls for row-block rb are emitted after
rb+1's sim matmuls so the PE never waits on ACT.

Engines: fp8 e4m3 inputs; each 512-col sim chunk is ONE DoubleRow matmul
(K=256 packed, ~2x bf16 PE rate).  Per 128-row block: ACT exponentiates
cols [1024:4096) (three 1024-col chunks -> SBUF bf16 + fused row-accum);
DVE handles [0:1024) and [4096:5120) with a bf16/u16 Schraudolph fast-exp
(+-4% per term, phase-averaged ~1e-5); gpsimd folds each Schraudolph tile
1024->512->256 (fp32 adds) so the DVE row-reduce touches only 256 cols.
gpsimd also does the positive-pair dots.  The host finishes with log/sum
in float64.  fp8 quantization dominates the error: ~9e-4 relative
(tolerance 2e-2).
"""

import numpy as np
import ml_dtypes

B = 4096
D = 256
N = 2 * B            # 8192 rows/cols of sim
NCORES = 8
RPC = N // NCORES    # 1024 rows per core
KCH = D // 128       # 2 contraction chunks of 128
NRB = RPC // 128     # 8 row-blocks of 128 per core
NCOL = 5120          # rotated columns computed per core (blocks j=0..4)
M_DEFAULT = 161.0    # logsumexp shift; safe while rowmax(2*h@h.T) in [M-70, M+79]
MASK_NEG = -1.0e9

# Schraudolph fast-exp constants, bf16/u16 variant:
#   bits16 = round(x * 2*A16 + (B16 - A16*M)), bitcast u16 -> bf16.
EXP_A16 = float(2 ** 7 / np.log(2.0))
EXP_B16 = 1064865216.0 / 65536.0

TRACE = False        # set True (e.g. from test.py) to request an NTFF trace
LAST_RESULTS = None  # BassKernelResults of the last run (for profiling)

_cache = {}


def _build():
    """Build the SPMD Bass/Tile program once per process."""
    if "nc" in _cache:
        return _cache["nc"]

    import concourse.tile as tile
    import concourse.mybir as mybir
    from concourse import bacc

    f32 = mybir.dt.float32
    bf16 = mybir.dt.bfloat16
    f8 = mybir.dt.float8e4
    u16 = mybir.dt.uint16
    DR = mybir.MatmulPerfMode.DoubleRow
    ALU = mybir.AluOpType
    AX = mybir.AxisListType.X

    nc = bacc.Bacc("TRN2", target_bir_lowering=False, num_devices=NCORES)
    CHUNKS = [(0, 512), (512, 1024), (1024, 2048),
              (2048, 3072), (3072, 4096), (4096, 5120)]
    ht_drams = [
        nc.dram_tensor(f"ht{i}", [128, KCH, c1 - c0], f8, kind="ExternalInput").ap()
        for i, (c0, c1) in enumerate(CHUNKS)
    ]
    bias_dram = nc.dram_tensor("biasm", [128, NRB], f32, kind="ExternalInput").ap()
    bias2_dram = nc.dram_tensor("bias2", [128, NRB], f32, kind="ExternalInput").ap()
    out_dram = nc.dram_tensor("out", [128, 44], f32, kind="ExternalOutput").ap()
    acc_dram = nc.dram_tensor("outacc", [6, 512], f32, kind="ExternalOutput").ap()

    with tile.TileContext(nc) as tc:
        with (
            tc.tile_pool(name="hpool", bufs=1) as hpool,
            tc.tile_pool(name="small", bufs=1) as small,
            tc.tile_pool(name="epool", bufs=6) as epool,
            tc.tile_pool(name="ipool", bufs=3) as ipool,
            tc.tile_pool(name="fpool", bufs=3) as fpool,
            tc.tile_pool(name="pschunk", bufs=3, space="PSUM") as pschunk,
            tc.tile_pool(name="psacc", bufs=1, space="PSUM") as psacc,
        ):
            # Tiny per-run bias tables on the gpsimd (SWDGE) queue.
            bias_sb = small.tile([128, NRB], f32)
            nc.gpsimd.dma_start(out=bias_sb, in_=bias_dram)
            bias2_sb = small.tile([128, NRB], f32)
            nc.gpsimd.dma_start(out=bias2_sb, in_=bias2_dram)

            ones_sb = small.tile([128, 1], bf16)
            nc.vector.memset(ones_sb, 1.0)

            # Device-generated diagonal patterns (beats DMAing 0.5MB of
            # masks): io[p, u] = u - 384 - p, so io == 0 on the shifted
            # diagonal.  eye = I (cols [384:512)); maskG holds -1e9 at
            # [p, 384 + p], and maskr slice v is its [384-128v, +512) window.
            io = small.tile([128, 1024], mybir.dt.int32)
            nc.gpsimd.iota(io, pattern=[[1, 1024]], base=-384, channel_multiplier=-1)
            eye_pos = small.tile([128, 128], bf16)
            nc.vector.tensor_scalar(
                eye_pos, io[:, 384:512], 0.0, 1.0,
                ALU.is_equal, ALU.mult,
            )
            maskG = small.tile([128, 896], bf16)
            nc.vector.tensor_scalar(
                maskG, io[:, 0:896], 0.0, MASK_NEG,
                ALU.is_equal, ALU.mult,
            )

            # Warm the ACT exp table during the DMA prologue.
            warm_sb = small.tile([128, 1], f32)
            nc.scalar.activation(
                out=warm_sb, in_=bias_sb[:, 0:1],
                func=mybir.ActivationFunctionType.Exp, bias=0.0, scale=0.0,
            )

            accA = psacc.tile([128, 512], f32, name="accA")
            accB = psacc.tile([128, 512], f32, name="accB")

            # Bridge the PE-idle window before the first h.T chunk lands so
            # the HAM clock gate sees sustained activity early.
            wsrc = small.tile([128, 128], bf16)
            nc.vector.memset(wsrc, 0.0)
            wps = pschunk.tile([128, 1024], f32, name="ps")
            for w in range(8):
                nc.tensor.matmul(
                    wps[:, (w % 4) * 128:(w % 4) * 128 + 128],
                    lhsT=wsrc, rhs=wsrc, start=True, stop=True,
                )

            # h.T in SBUF in consumption order; each chunk is stored
            # [p][k][c]-contiguous in DRAM (2KB-per-partition runs), split
            # across the sync and gpsimd DMA queues for 2x arrival rate.
            col_ranges = CHUNKS
            ht_tiles = []
            for i, (c0, c1) in enumerate(col_ranges):
                t = hpool.tile([128, KCH, c1 - c0], f8, name=f"ht_{c0}")
                eng = nc.sync if i < 3 else nc.gpsimd
                eng.dma_start(out=t, in_=ht_drams[i])
                ht_tiles.append(t)

            def rhs_slice(c0, w=512):
                """[128, 2, w] slice of rotated h.T at global column c0."""
                for (r0, r1), t in zip(col_ranges, ht_tiles):
                    if r0 <= c0 < r1:
                        assert c0 + w <= r1
                        return t[:, :, c0 - r0:c0 - r0 + w]
                raise AssertionError(c0)

            def lhsT_dr(rb):
                """[128, 2, 128] row-block weights (columns rb*128..+128)."""
                t = ht_tiles[0] if rb < 4 else ht_tiles[1]
                o = (rb % 4) * 128
                return t[:, :, o:o + 128]

            res_sb = small.tile([128, 44], f32)

            def sim_mms(rb, c0, ps, mask=False):
                """Two DoubleRow matmuls filling ps[128,1024] = sim cols
                [c0, c0+1024); optionally accumulate the diag mask."""
                cs0 = rb // 4
                for cs in range(2):
                    nc.tensor.matmul(
                        ps[:, cs * 512:(cs + 1) * 512],
                        lhsT=lhsT_dr(rb), rhs=rhs_slice(c0 + cs * 512),
                        start=True, stop=not (mask and cs == cs0),
                        perf_mode=DR,
                    )
                if mask:
                    v = rb % 4
                    nc.tensor.matmul(
                        ps[:, cs0 * 512:(cs0 + 1) * 512],
                        lhsT=eye_pos, rhs=maskG[:, 384 - 128 * v:896 - 128 * v],
                        start=False, stop=True,
                    )

            def emit_D(rb, c0, rescol):
                # Schraudolph chunk: DVE fast-exp bits, gpsimd folds
                # 1024->512->256 in fp32, DVE row-reduces the 256.
                ps = pschunk.tile([128, 1024], f32, name="ps")
                sim_mms(rb, c0, ps, mask=c0 == 0)
                ti = ipool.tile([128, 1024], u16, name="ti")
                nc.vector.tensor_scalar(
                    ti, ps, 2.0 * EXP_A16, bias2_sb[:, rb:rb + 1],
                    ALU.mult, ALU.add,
                )
                tb = ti.bitcast(bf16)
                f1 = fpool.tile([128, 512], f32, name="f1")
                nc.gpsimd.tensor_add(f1, tb[:, 0:512], tb[:, 512:1024])
                nc.vector.reduce_sum(res_sb[:, rescol:rescol + 1], f1, axis=AX)

            def emit_A(rb, c0, rescol):
                # ACT chunk: exp -> SBUF bf16 (feeds the delayed column-sum
                # matmuls) + fused row-accum.
                ps = pschunk.tile([128, 1024], f32, name="ps")
                sim_mms(rb, c0, ps)
                eA = epool.tile([128, 1024], bf16, name="eA")
                nc.scalar.activation(
                    out=eA, in_=ps,
                    func=mybir.ActivationFunctionType.Exp,
                    bias=bias_sb[:, rb:rb + 1], scale=2.0,
                    accum_out=res_sb[:, rescol:rescol + 1],
                )
                return eA

            def emit_colsums(rb, eAs):
                # Column sums of exp for cols [1024:4096) of row-block rb:
                # six ones.T @ eA matmuls accumulating into accA (4 slots)
                # and accB (2 slots); concurrent within distinct col-groups.
                for g in range(6):
                    acc, q = (accA, g) if g < 4 else (accB, g - 4)
                    nc.tensor.matmul(
                        acc[32 * q:32 * q + 1, :],
                        lhsT=ones_sb,
                        rhs=eAs[g // 2][:, (g % 2) * 512:(g % 2) * 512 + 512],
                        start=rb == 0, stop=rb == NRB - 1,
                        tile_position=(0, 32 * q), skip_group_check=True,
                    )

            def emit_posdot():
                # Positive-pair partial dots: rotated cols [0:1024) are this
                # core's rows, [4096:5120) their partners.  gpsimd multiply
                # and fold, tiny DVE row-reduce.
                for k in range(KCH):
                    for half in range(2):
                        pp = small.tile([128, 512], f32, name=f"pp_{k}_{half}")
                        nc.gpsimd.tensor_mul(
                            pp,
                            ht_tiles[half][:, k, :],
                            ht_tiles[5][:, k, half * 512:(half + 1) * 512],
                        )
                        nc.vector.reduce_sum(
                            res_sb[:, 40 + 2 * k + half:41 + 2 * k + half],
                            pp, axis=AX,
                        )

            prev_eAs = None
            for rb in range(NRB):
                emit_D(rb, 0, rb * 5 + 0)
                eA1 = emit_A(rb, 1024, rb * 5 + 1)
                if prev_eAs is not None:
                    emit_colsums(rb - 1, prev_eAs)
                eA2 = emit_A(rb, 2048, rb * 5 + 2)
                eA3 = emit_A(rb, 3072, rb * 5 + 3)
                if rb == 2:
                    emit_posdot()
                emit_D(rb, 4096, rb * 5 + 4)
                prev_eAs = (eA1, eA2, eA3)
            emit_colsums(NRB - 1, prev_eAs)

            # Ship rb0-6 partials + posdots while rb7 still computes.
            nc.sync.dma_start(out=out_dram[:, 0:35], in_=res_sb[:, 0:35])

            # Column-sum accumulators: PSUM -> SBUF (DMA cannot read PSUM),
            # then one tiny DMA of the 6 used partition rows.
            accA_sb = small.tile([128, 512], f32)
            accB_sb = small.tile([128, 512], f32)
            nc.scalar.copy(accA_sb, accA)
            nc.vector.tensor_copy(accB_sb, accB)
            nc.sync.dma_start(out=acc_dram[0:4, :], in_=accA_sb[0:128:32, :])
            nc.sync.dma_start(out=acc_dram[4:6, :], in_=accB_sb[0:64:32, :])
            nc.sync.dma_start(out=out_dram[:, 35:44], in_=res_sb[:, 35:44])

    nc.compile()
    _cache["nc"] = nc
    return nc


_CHUNKS = [(0, 512), (512, 1024), (1024, 2048),
           (2048, 3072), (3072, 4096), (4096, 5120)]


def _make_static_inputs(h_i, h_j):
    """Per-core rotated h.T (fp8 e4m3), cols [0:5120), one contiguous
    [128, 2, width] array per DMA chunk."""
    h = np.concatenate([np.asarray(h_i), np.asarray(h_j)], axis=0).astype(np.float32)
    hT = np.ascontiguousarray(h.T)  # [256, 8192]
    hts = []
    for c in range(NCORES):
        htc = np.roll(hT, -RPC * c, axis=1)[:, :NCOL].astype(ml_dtypes.float8_e4m3)
        h3 = htc.reshape(KCH, 128, NCOL)
        hts.append([
            np.ascontiguousarray(h3[:, :, c0:c1].transpose(1, 0, 2))
            for c0, c1 in _CHUNKS
        ])
    return hts


def _axon_reset():
    try:
        import ctypes

        lib = ctypes.CDLL("/opt/axon/libaxon_pjrt.so")
        lib.axon_reset.restype = ctypes.c_int64
        return lib.axon_reset() == 0
    except Exception:
        return False


def _run(nc, hts, M):
    global LAST_RESULTS
    from concourse import bass_utils

    biasm = np.full((128, NRB), -M, dtype=np.float32)
    bias2 = np.full((128, NRB), EXP_B16 - EXP_A16 * M, dtype=np.float32)
    in_maps = [
        {
            **{f"ht{i}": hts[c][i] for i in range(6)},
            "biasm": biasm,
            "bias2": bias2,
        }
        for c in range(NCORES)
    ]
    try:
        results = bass_utils.run_bass_kernel_spmd(
            nc, in_maps, core_ids=list(range(NCORES)), trace=TRACE
        )
    except Exception:
        if not _axon_reset():
            raise
        results = bass_utils.run_bass_kernel_spmd(
            nc, in_maps, core_ids=list(range(NCORES)), trace=TRACE
        )
    LAST_RESULTS = results
    return results.results


def _host_fallback(h_i, h_j):
    """Exact float64 evaluation on the host (safety net for data far
    outside the M window; never triggered by in-distribution inputs)."""
    h = np.concatenate([np.asarray(h_i), np.asarray(h_j)], 0).astype(np.float64)
    sim = 2.0 * (h @ h.T)
    np.fill_diagonal(sim, -np.inf)
    m = sim.max(1)
    lse = m + np.log(np.exp(sim - m[:, None]).sum(1))
    pos = np.concatenate([2.0 * (h[:B] * h[B:]).sum(1)] * 2)
    return np.float32((lse - pos).mean())


def kernel(h_i, h_j):
    nc = _build()
    hts = _make_static_inputs(h_i, h_j)

    for attempt, M in enumerate([M_DEFAULT, M_DEFAULT - 60.0, M_DEFAULT + 60.0]):
        res = _run(nc, hts, M)
        # Assemble per-row exp sums: own row partials + mirror column sums.
        S = np.zeros(N)
        total_pd = 0.0
        for c in range(NCORES):
            out = res[c]["out"].astype(np.float64)
            acc = res[c]["outacc"].astype(np.float64)
            own = out[:, :40].reshape(128, NRB, 5).sum(axis=2)  # [p, rb]
            rows = (RPC * c + np.arange(RPC)) % N
            S[rows] += own.T.reshape(RPC)
            for q in range(6):
                j0 = 1024 + 512 * q
                tgt = (RPC * c + j0 + np.arange(512)) % N
                S[tgt] += acc[q]
            total_pd += out[:, 40:44].sum()
        if np.all(np.isfinite(S) & (S > 0.0)):
            total_lse = N * M + np.log(S).sum()
            loss = (total_lse - 2.0 * total_pd) / float(N)
            return np.array(loss, dtype=np.float32)

    return _host_fallback(h_i, h_j)


if __name__ == "__main__":
    rng = np.random.default_rng(0)
    h_i = rng.standard_normal((B, D), dtype=np.float32)
    h_j = rng.standard_normal((B, D), dtype=np.float32)
    print("loss:", kernel(h_i, h_j))


# revision 13
# speedup vs baseline: 2.0956x; 1.0518x over previous
"""NT-Xent contrastive loss (forward) on 8 TRN2 NeuronCores via Bass/Tile.

Math: with h = concat(h_i, h_j) [N=8192, D=256], sim = (h @ h.T) / 0.5,
loss = mean_r( logsumexp_j(sim[r, j], j != r) - pos_r ), where
pos_r = sim[r, partner(r)] = 2 * h_i[q] . h_j[q].  The loss separates:
loss = (sum_r lse_r - sum_r pos_r) / N, and sum_r pos_r = 4 * sum(h_i * h_j).

Sharding + symmetry: core c owns rows [1024c, 1024c+1024).  sim is
symmetric, so each core computes only 5 of the 8 column-blocks of its row
stripe (rotated columns [0, 5120)): block j=0 is its own diagonal block
(rowsums only, diagonal masked), blocks j=1..3 produce BOTH per-row sums
(this core's rows) and per-column sums (the mirror cores' rows; host
assembles S_r from all cores), and block j=4 is computed by both cores of
its pair, rowsums only.  Column sums cost no ACT/DVE work: the PE
accumulates ones.T @ exp(tile) into persistent PSUM accumulators across
all 8 row-blocks, 4 accumulators packed per PSUM bank via 32-partition
col-tiling (4 such matmuls run concurrently in distinct array
col-groups).  The column-sum matmuls for row-block rb are emitted after
rb+1's sim matmuls so the PE never waits on ACT.

Engines: fp8 e4m3 inputs; each 512-col sim chunk is ONE DoubleRow matmul
(K=256 packed, ~2x bf16 PE rate).  Per 128-row block: ACT exponentiates
cols [1024:4096) (three 1024-col chunks -> SBUF bf16 + fused row-accum);
DVE handles [0:1024) and [4096:5120) with a bf16/u16 Schraudolph fast-exp
(+-4% per term, phase-averaged ~1e-5); gpsimd folds each Schraudolph tile
1024->512->256 (fp32 adds) so the DVE row-reduce touches only 256 cols.
gpsimd also does the positive-pair dots.  The host finishes with log/sum
in float64.  fp8 quantization dominates the error: ~9e-4 relative
(tolerance 2e-2).
"""

import numpy as np
import ml_dtypes

B = 4096
D = 256
N = 2 * B            # 8192 rows/cols of sim
NCORES = 8
RPC = N // NCORES    # 1024 rows per core
KCH = D // 128       # 2 contraction chunks of 128
NRB = RPC // 128     # 8 row-blocks of 128 per core
NCOL = 5120          # rotated columns computed per core (blocks j=0..4)
M_DEFAULT = 161.0    # logsumexp shift; safe while rowmax(2*h@h.T) in [M-70, M+79]
MASK_NEG = -1.0e9

# Schraudolph fast-exp constants, bf16/u16 variant:
#   bits16 = round(x * 2*A16 + (B16 - A16*M)), bitcast u16 -> bf16.
EXP_A16 = float(2 ** 7 / np.log(2.0))
EXP_B16 = 1064865216.0 / 65536.0

TRACE = False        # set True (e.g. from test.py) to request an NTFF trace
LAST_RESULTS = None  # BassKernelResults of the last run (for profiling)

_cache = {}


def _build():
    """Build the SPMD Bass/Tile program once per process."""
    if "nc" in _cache:
        return _cache["nc"]

    import concourse.tile as tile
    import concourse.mybir as mybir
    from concourse import bacc

    f32 = mybir.dt.float32
    bf16 = mybir.dt.bfloat16
    f8 = mybir.dt.float8e4
    u16 = mybir.dt.uint16
    DR = mybir.MatmulPerfMode.DoubleRow
    ALU = mybir.AluOpType
    AX = mybir.AxisListType.X

    nc = bacc.Bacc("TRN2", target_bir_lowering=False, num_devices=NCORES)
    CHUNKS = [(0, 512), (512, 1024), (1024, 2048),
              (2048, 3072), (3072, 4096), (4096, 5120)]
    ht_drams = [
        nc.dram_tensor(f"ht{i}", [128, KCH, c1 - c0], f8, kind="ExternalInput").ap()
        for i, (c0, c1) in enumerate(CHUNKS)
    ]
    bias_dram = nc.dram_tensor("biasm", [128, NRB], f32, kind="ExternalInput").ap()
    bias2_dram = nc.dram_tensor("bias2", [128, NRB], f32, kind="ExternalInput").ap()
    out_dram = nc.dram_tensor("out", [128, 44], f32, kind="ExternalOutput").ap()
    acc_dram = nc.dram_tensor("outacc", [6, 512], f32, kind="ExternalOutput").ap()

    with tile.TileContext(nc) as tc:
        with (
            tc.tile_pool(name="hpool", bufs=1) as hpool,
            tc.tile_pool(name="small", bufs=1) as small,
            tc.tile_pool(name="epool", bufs=6) as epool,
            tc.tile_pool(name="ipool", bufs=3) as ipool,
            tc.tile_pool(name="fpool", bufs=3) as fpool,
            tc.tile_pool(name="pschunk", bufs=3, space="PSUM") as pschunk,
            tc.tile_pool(name="psacc", bufs=1, space="PSUM") as psacc,
        ):
            # Tiny per-run bias tables on the gpsimd (SWDGE) queue.
            bias_sb = small.tile([128, NRB], f32)
            nc.gpsimd.dma_start(out=bias_sb, in_=bias_dram)
            bias2_sb = small.tile([128, NRB], f32)
            nc.gpsimd.dma_start(out=bias2_sb, in_=bias2_dram)

            ones_sb = small.tile([128, 1], bf16)
            nc.vector.memset(ones_sb, 1.0)

            # Device-generated diagonal patterns (beats DMAing 0.5MB of
            # masks): io[p, u] = u - 384 - p, so io == 0 on the shifted
            # diagonal.  eye = I (cols [384:512)); maskG holds -1e9 at
            # [p, 384 + p], and maskr slice v is its [384-128v, +512) window.
            io = small.tile([128, 1024], mybir.dt.int32)
            nc.gpsimd.iota(io, pattern=[[1, 1024]], base=-384, channel_multiplier=-1)
            eye_pos = small.tile([128, 128], bf16)
            nc.vector.tensor_scalar(
                eye_pos, io[:, 384:512], 0.0, 1.0,
                ALU.is_equal, ALU.mult,
            )
            maskG = small.tile([128, 896], bf16)
            nc.vector.tensor_scalar(
                maskG, io[:, 0:896], 0.0, MASK_NEG,
                ALU.is_equal, ALU.mult,
            )

            # Warm the ACT exp table during the DMA prologue.
            warm_sb = small.tile([128, 1], f32)
            nc.scalar.activation(
                out=warm_sb, in_=bias_sb[:, 0:1],
                func=mybir.ActivationFunctionType.Exp, bias=0.0, scale=0.0,
            )

            accA = psacc.tile([128, 512], f32, name="accA")
            accB = psacc.tile([128, 512], f32, name="accB")

            # Bridge the PE-idle window before the first h.T chunk lands so
            # the HAM clock gate sees sustained activity early.
            wsrc = small.tile([128, 128], bf16)
            nc.vector.memset(wsrc, 0.0)
            wps = pschunk.tile([128, 1024], f32, name="ps")
            for w in range(20):
                nc.tensor.matmul(
                    wps[:, (w % 4) * 128:(w % 4) * 128 + 128],
                    lhsT=wsrc, rhs=wsrc, start=True, stop=True,
                )

            # h.T in SBUF in consumption order; each chunk is stored
            # [p][k][c]-contiguous in DRAM (2KB-per-partition runs), split
            # across the sync and gpsimd DMA queues for 2x arrival rate.
            col_ranges = CHUNKS
            ht_tiles = []
            qmap = {0: nc.sync, 1: nc.scalar, 2: nc.sync,
                    3: nc.gpsimd, 4: nc.scalar, 5: nc.gpsimd}
            for i, (c0, c1) in enumerate(col_ranges):
                t = hpool.tile([128, KCH, c1 - c0], f8, name=f"ht_{c0}")
                qmap[i].dma_start(out=t, in_=ht_drams[i])
                ht_tiles.append(t)

            def rhs_slice(c0, w=512):
                """[128, 2, w] slice of rotated h.T at global column c0."""
                for (r0, r1), t in zip(col_ranges, ht_tiles):
                    if r0 <= c0 < r1:
                        assert c0 + w <= r1
                        return t[:, :, c0 - r0:c0 - r0 + w]
                raise AssertionError(c0)

            def lhsT_dr(rb):
                """[128, 2, 128] row-block weights (columns rb*128..+128)."""
                t = ht_tiles[0] if rb < 4 else ht_tiles[1]
                o = (rb % 4) * 128
                return t[:, :, o:o + 128]

            res_sb = small.tile([128, 44], f32)

            def sim_mms(rb, c0, ps, mask=False):
                """Two DoubleRow matmuls filling ps[128,1024] = sim cols
                [c0, c0+1024); optionally accumulate the diag mask."""
                cs0 = rb // 4
                for cs in range(2):
                    nc.tensor.matmul(
                        ps[:, cs * 512:(cs + 1) * 512],
                        lhsT=lhsT_dr(rb), rhs=rhs_slice(c0 + cs * 512),
                        start=True, stop=not (mask and cs == cs0),
                        perf_mode=DR,
                    )
                if mask:
                    v = rb % 4
                    nc.tensor.matmul(
                        ps[:, cs0 * 512:(cs0 + 1) * 512],
                        lhsT=eye_pos, rhs=maskG[:, 384 - 128 * v:896 - 128 * v],
                        start=False, stop=True,
                    )

            def emit_D(rb, c0, rescol):
                # Schraudolph chunk: DVE fast-exp bits, gpsimd folds
                # 1024->512->256 in fp32, DVE row-reduces the 256.
                ps = pschunk.tile([128, 1024], f32, name="ps")
                sim_mms(rb, c0, ps, mask=c0 == 0)
                ti = ipool.tile([128, 1024], u16, name="ti")
                nc.vector.tensor_scalar(
                    ti, ps, 2.0 * EXP_A16, bias2_sb[:, rb:rb + 1],
                    ALU.mult, ALU.add,
                )
                tb = ti.bitcast(bf16)
                f1 = fpool.tile([128, 512], f32, name="f1")
                nc.gpsimd.tensor_add(f1, tb[:, 0:512], tb[:, 512:1024])
                nc.vector.reduce_sum(res_sb[:, rescol:rescol + 1], f1, axis=AX)

            def emit_A(rb, c0, rescol):
                # ACT chunk: exp -> SBUF bf16 (feeds the delayed column-sum
                # matmuls) + fused row-accum.
                ps = pschunk.tile([128, 1024], f32, name="ps")
                sim_mms(rb, c0, ps)
                eA = epool.tile([128, 1024], bf16, name="eA")
                nc.scalar.activation(
                    out=eA, in_=ps,
                    func=mybir.ActivationFunctionType.Exp,
                    bias=bias_sb[:, rb:rb + 1], scale=2.0,
                    accum_out=res_sb[:, rescol:rescol + 1],
                )
                return eA

            def emit_colsums(rb, eAs):
                # Column sums of exp for cols [1024:4096) of row-block rb:
                # six ones.T @ eA matmuls accumulating into accA (4 slots)
                # and accB (2 slots); concurrent within distinct col-groups.
                for g in range(6):
                    acc, q = (accA, g) if g < 4 else (accB, g - 4)
                    nc.tensor.matmul(
                        acc[32 * q:32 * q + 1, :],
                        lhsT=ones_sb,
                        rhs=eAs[g // 2][:, (g % 2) * 512:(g % 2) * 512 + 512],
                        start=rb == 0, stop=rb == NRB - 1,
                        tile_position=(0, 32 * q), skip_group_check=True,
                    )

            def emit_posdot():
                # Positive-pair partial dots: rotated cols [0:1024) are this
                # core's rows, [4096:5120) their partners.  gpsimd multiply
                # and fold, tiny DVE row-reduce.
                for k in range(KCH):
                    for half in range(2):
                        pp = small.tile([128, 512], f32, name=f"pp_{k}_{half}")
                        nc.gpsimd.tensor_mul(
                            pp,
                            ht_tiles[half][:, k, :],
                            ht_tiles[5][:, k, half * 512:(half + 1) * 512],
                        )
                        nc.vector.reduce_sum(
                            res_sb[:, 40 + 2 * k + half:41 + 2 * k + half],
                            pp, axis=AX,
                        )

            prev_eAs = None
            for rb in range(NRB):
                if rb == 0:
                    # First row-block consumes chunks in DMA-arrival order.
                    emit_D(rb, 0, rb * 5 + 0)
                    eA1 = emit_A(rb, 1024, rb * 5 + 1)
                    eA2 = emit_A(rb, 2048, rb * 5 + 2)
                    eA3 = emit_A(rb, 3072, rb * 5 + 3)
                    emit_D(rb, 4096, rb * 5 + 4)
                else:
                    eA1 = emit_A(rb, 1024, rb * 5 + 1)
                    if prev_eAs is not None:
                        emit_colsums(rb - 1, prev_eAs)
                    eA2 = emit_A(rb, 2048, rb * 5 + 2)
                    emit_D(rb, 0, rb * 5 + 0)
                    eA3 = emit_A(rb, 3072, rb * 5 + 3)
                    if rb == 2:
                        emit_posdot()
                    emit_D(rb, 4096, rb * 5 + 4)
                prev_eAs = (eA1, eA2, eA3)
            emit_colsums(NRB - 1, prev_eAs)

            # Ship rb0-6 partials + posdots while rb7 still computes.
            nc.sync.dma_start(out=out_dram[:, 0:35], in_=res_sb[:, 0:35])

            # Column-sum accumulators: PSUM -> SBUF (DMA cannot read PSUM),
            # then one tiny DMA of the 6 used partition rows.
            accA_sb = small.tile([128, 512], f32)
            accB_sb = small.tile([128, 512], f32)
            nc.scalar.copy(accA_sb, accA)
            nc.vector.tensor_copy(accB_sb, accB)
            nc.sync.dma_start(out=acc_dram[0:4, :], in_=accA_sb[0:128:32, :])
            nc.sync.dma_start(out=acc_dram[4:6, :], in_=accB_sb[0:64:32, :])
            nc.sync.dma_start(out=out_dram[:, 35:44], in_=res_sb[:, 35:44])

    nc.compile()
    _cache["nc"] = nc
    return nc


_CHUNKS = [(0, 512), (512, 1024), (1024, 2048),
           (2048, 3072), (3072, 4096), (4096, 5120)]


def _make_static_inputs(h_i, h_j):
    """Per-core rotated h.T (fp8 e4m3), cols [0:5120), one contiguous
    [128, 2, width] array per DMA chunk."""
    h = np.concatenate([np.asarray(h_i), np.asarray(h_j)], axis=0).astype(np.float32)
    hT = np.ascontiguousarray(h.T)  # [256, 8192]
    hts = []
    for c in range(NCORES):
        htc = np.roll(hT, -RPC * c, axis=1)[:, :NCOL].astype(ml_dtypes.float8_e4m3)
        h3 = htc.reshape(KCH, 128, NCOL)
        hts.append([
            np.ascontiguousarray(h3[:, :, c0:c1].transpose(1, 0, 2))
            for c0, c1 in _CHUNKS
        ])
    return hts


def _axon_reset():
    try:
        import ctypes

        lib = ctypes.CDLL("/opt/axon/libaxon_pjrt.so")
        lib.axon_reset.restype = ctypes.c_int64
        return lib.axon_reset() == 0
    except Exception:
        return False


def _run(nc, hts, M):
    global LAST_RESULTS
    from concourse import bass_utils

    biasm = np.full((128, NRB), -M, dtype=np.float32)
    bias2 = np.full((128, NRB), EXP_B16 - EXP_A16 * M, dtype=np.float32)
    in_maps = [
        {
            **{f"ht{i}": hts[c][i] for i in range(6)},
            "biasm": biasm,
            "bias2": bias2,
        }
        for c in range(NCORES)
    ]
    try:
        results = bass_utils.run_bass_kernel_spmd(
            nc, in_maps, core_ids=list(range(NCORES)), trace=TRACE
        )
    except Exception:
        if not _axon_reset():
            raise
        results = bass_utils.run_bass_kernel_spmd(
            nc, in_maps, core_ids=list(range(NCORES)), trace=TRACE
        )
    LAST_RESULTS = results
    return results.results


def _host_fallback(h_i, h_j):
    """Exact float64 evaluation on the host (safety net for data far
    outside the M window; never triggered by in-distribution inputs)."""
    h = np.concatenate([np.asarray(h_i), np.asarray(h_j)], 0).astype(np.float64)
    sim = 2.0 * (h @ h.T)
    np.fill_diagonal(sim, -np.inf)
    m = sim.max(1)
    lse = m + np.log(np.exp(sim - m[:, None]).sum(1))
    pos = np.concatenate([2.0 * (h[:B] * h[B:]).sum(1)] * 2)
    return np.float32((lse - pos).mean())


def kernel(h_i, h_j):
    nc = _build()
    hts = _make_static_inputs(h_i, h_j)

    for attempt, M in enumerate([M_DEFAULT, M_DEFAULT - 60.0, M_DEFAULT + 60.0]):
        res = _run(nc, hts, M)
        # Assemble per-row exp sums: own row partials + mirror column sums.
        S = np.zeros(N)
        total_pd = 0.0
        for c in range(NCORES):
            out = res[c]["out"].astype(np.float64)
            acc = res[c]["outacc"].astype(np.float64)
            own = out[:, :40].reshape(128, NRB, 5).sum(axis=2)  # [p, rb]
            rows = (RPC * c + np.arange(RPC)) % N
            S[rows] += own.T.reshape(RPC)
            for q in range(6):
                j0 = 1024 + 512 * q
                tgt = (RPC * c + j0 + np.arange(512)) % N
                S[tgt] += acc[q]
            total_pd += out[:, 40:44].sum()
        if np.all(np.isfinite(S) & (S > 0.0)):
            total_lse = N * M + np.log(S).sum()
            loss = (total_lse - 2.0 * total_pd) / float(N)
            return np.array(loss, dtype=np.float32)

    return _host_fallback(h_i, h_j)


if __name__ == "__main__":
    rng = np.random.default_rng(0)
    h_i = rng.standard_normal((B, D), dtype=np.float32)
    h_j = rng.standard_normal((B, D), dtype=np.float32)
    print("loss:", kernel(h_i, h_j))
